# revision 8
# baseline (speedup 1.0000x reference)
"""MoE (8 experts, top-2, SwiGLU) Trainium2 kernel.

Strategy (expert-parallel + overflow H-split): the router is computed on
host as part of the sharding step.  Phase 1: core e runs expert e's
SwiGLU over the first C1 tokens assigned to expert e (C1 = mean load,
tile-rounded), scaling rows by the renormalized routing weight.  Phase
2 balances the remainder: every core processes ALL experts' overflow
tokens over its own H/8 row-slice of the expert weights (slices loaded
once, 1.5 MB SBUF), producing partial outputs the host sums.  This
removes the load-imbalance capacity padding (max load vs mean) that
would otherwise idle 7 cores while the heaviest expert finishes.

Under sustained 8-core load the PE clock throttles to ~1.95 GHz, and
the baseline measures as pure column-streaming at that clock, so the
only wins available are column-count reductions (balance) and
energy/overhead trims (y output in bf16, x loads triggered from the
idle gpsimd queue so block boundaries never stall the PE).

Device kernel (bf16 operands, fp32 PSUM accumulation, rel err ~4e-3):
all weights live SBUF-resident in bf16, loaded once in partition-major
contiguous layout.  Per 512-token block:
    g^T = Wg^T.T @ x^T   [H, c]   (d contraction, h on partitions)
    u^T = Wu^T.T @ x^T   [H, c]
    hid = silu(g^T) * u^T          (bf16 SBUF)
    y   = hid.T @ Wd^T   [c, D]   (h contraction, PSUM-chained,
                                   tokens on partitions)
    y  *= w_scale[token]           (per-partition scalar on ACT engine)
"""

import sys

for _p in ("/opt/trn_rl_repo", "/root/.axon_site/_ro/trn_rl_repo"):
    if _p not in sys.path:
        sys.path.append(_p)

import numpy as np

import concourse.bacc as bacc
import concourse.bass_utils as _bass_utils
import concourse.mybir as mybir
from concourse import tile
from concourse.bass_utils import run_bass_kernel_spmd

# birsim (walrus's in-compiler simulator) costs ~85s on this program and
# only re-verifies what the correctness test already covers; skip it.
LDW_OPT = False

if not getattr(_bass_utils, "_moe_birsim_patch", False):
    _bass_utils._moe_birsim_patch = True
    _orig_run_command = _bass_utils.run_command

    def _run_command_no_birsim(argv, **kw):
        argv = ["--enable-birsim=false" if a == "--enable-birsim=true" else a
                for a in argv]
        if LDW_OPT:
            argv = ["--enable-ldw-opt=true" if a == "--enable-ldw-opt=false"
                    else a for a in argv]
        return _orig_run_command(argv, **kw)

    _bass_utils.run_command = _run_command_no_birsim

B, S, D, H, E, TOPK = 4, 2048, 1024, 2048, 8, 2
T = B * S
N_CORES = 8
P = 128
ND = D // P   # 8 d-tiles
NH = H // P   # 16 h-tiles
HS = H // N_CORES          # 256: phase-2 per-core H slice
NJ2 = HS // P              # 2 j-tiles per expert in phase 2
F32 = mybir.dt.float32
BF16 = mybir.dt.bfloat16


def build_nc(key, repeat: int = 1):
    """Build the SPMD Bass program.

    key = (C1, slots): C1 = phase-1 per-core capacity (multiple of 128);
    slots = per-expert phase-2 slot sizes (multiples of 128, may be 0).
    """
    C1, slots = key
    assert C1 % P == 0
    NT1 = C1 // P
    C2 = sum(slots)
    # ovf: per overflow-expert (dense index, x2 col offset, token count)
    ovf = [(e, off, n) for e, off, n in
           zip(range(E), np.cumsum([0] + list(slots))[:-1], slots) if n]
    NO = len(ovf)
    # down tiles: (dense expert idx, x2/y2 row offset, token count <= 128)
    tiles2 = []
    for gi, (e, off, n) in enumerate(ovf):
        t = 0
        while t < n:
            tiles2.append((gi, off + t, min(P, n - t)))
            t += P
    NTL2 = len(tiles2)

    nc = bacc.Bacc("TRN2", target_bir_lowering=False, debug=False,
                   num_devices=N_CORES)
    # layouts chosen so every matmul chain reads SBUF contiguously
    # ascending (strided reads over >8KB/partition footprints measure
    # ~30-45 ns/MM slower than contiguous sweeps on the PE read ports)
    x_t = nc.dram_tensor("x_t", [P, ND, C1], BF16, kind="ExternalInput")
    wgu_t = nc.dram_tensor("wgu_t", [P, NH, 2, ND, P], BF16,
                           kind="ExternalInput")
    wd_t = nc.dram_tensor("wd_t", [P, 2, NH, 512], BF16, kind="ExternalInput")
    wsc = nc.dram_tensor("wsc", [P, NT1], F32, kind="ExternalInput")
    y = nc.dram_tensor("y", [C1, D], BF16, kind="ExternalOutput")
    if C2:
        x2_t = nc.dram_tensor("x2_t", [P, ND, C2], BF16,
                              kind="ExternalInput")
        w2gu = nc.dram_tensor("w2gu", [P, NO, NJ2, 2, ND, P], BF16,
                              kind="ExternalInput")
        w2d = nc.dram_tensor("w2d", [P, NO, NJ2, 2, 512], BF16,
                             kind="ExternalInput")
        wsc2 = nc.dram_tensor("wsc2", [P, NTL2], F32, kind="ExternalInput")
        y2 = nc.dram_tensor("y2", [C2, D], F32, kind="ExternalOutput")

    # phase-1 token blocks of up to 4 tiles (512 tokens)
    blocks = []
    off = 0
    while off < NT1:
        bt = min(4, NT1 - off)
        blocks.append((off * P, bt * P))
        off += bt

    with tile.TileContext(nc) as tc:
        with (
            tc.tile_pool(name="wgp", bufs=1) as wgp,
            tc.tile_pool(name="wdp", bufs=1) as wdp,
            tc.tile_pool(name="cp", bufs=1) as cp,
            tc.tile_pool(name="xp", bufs=3) as xp,
            tc.tile_pool(name="hp", bufs=2) as hp,
            tc.tile_pool(name="op", bufs=4) as op,
            tc.tile_pool(name="pg", bufs=3, space="PSUM") as pgp,
            tc.tile_pool(name="pu", bufs=3, space="PSUM") as pup,
            tc.tile_pool(name="py", bufs=2, space="PSUM") as pyp,
        ):
            # weight preload split per-j on the SP DMA ring
            wsc_tile = cp.tile([P, NT1], F32, tag="wsc")
            nc.sync.dma_start(wsc_tile[:], wsc.ap())
            wgut = wgp.tile([P, NH, 2, ND, P], BF16, tag="wgut")
            for j in range(NH):
                nc.sync.dma_start(wgut[:, j], wgu_t.ap()[:, j])
            wdt = wdp.tile([P, 2, NH, 512], BF16, tag="wdt")
            for dh in range(2):
                nc.sync.dma_start(wdt[:, dh], wd_t.ap()[:, dh])
            if C2:
                wsc2_tile = cp.tile([P, NTL2], F32, tag="wsc2")
                nc.sync.dma_start(wsc2_tile[:], wsc2.ap())
                w2gut = wgp.tile([P, NO, NJ2, 2, ND, P], BF16, tag="w2gut")
                nc.sync.dma_start(w2gut[:], w2gu.ap())
                w2dt = wdp.tile([P, NO, NJ2, 2, 512], BF16, tag="w2dt")
                nc.sync.dma_start(w2dt[:], w2d.ap())

            def load_x(boff, bs):
                # gpsimd queue is otherwise idle, so the trigger fires as
                # soon as the pool buffer frees (deep prefetch), never
                # stalling the PE at block boundaries
                xc = xp.tile([P, ND, 512], BF16, tag="xc", name=f"xc{boff}")
                nc.gpsimd.dma_start(xc[:, :, :bs],
                                    x_t.ap()[:, :, boff:boff + bs])
                return xc

            def gateup_grp(grp, xcs, hds):
                # each stationary feeds one matmul per block in grp
                # back-to-back — the LDWEIGHTS hides behind the other
                # blocks' stream time
                for j in range(NH):
                    sil = None
                    for gu, pool in ((0, pgp), (1, pup)):
                        ps = [pool.tile([P, 512], F32, tag=pool.name,
                                        name=f"{pool.name}{gi}")
                              for gi in range(len(grp))]
                        for d in range(ND):
                            for gi, (boff, bs) in enumerate(grp):
                                nc.tensor.matmul(ps[gi][:, :bs],
                                                 wgut[:, j, gu, d, :],
                                                 xcs[gi][:, d, :bs],
                                                 start=(d == 0),
                                                 stop=(d == ND - 1))
                        if gu == 0:
                            sil = ps
                        else:
                            for gi, (boff, bs) in enumerate(grp):
                                bt = bs // P
                                nc.scalar.activation(
                                    hds[gi][:, :bt, j, :], sil[gi][:, :bs],
                                    mybir.ActivationFunctionType.Silu)
                                nc.vector.tensor_mul(hds[gi][:, :bt, j, :],
                                                     hds[gi][:, :bt, j, :],
                                                     ps[gi][:, :bs])

            def down(hid, boff, bt):
                # down: dh0/dh1 chains interleaved per j so each hid
                # stationary feeds two back-to-back matmuls — hides the
                # LDWEIGHTS behind 2x the stream time
                for tt in range(bt):
                    abs_tt = boff // P + tt
                    py0 = pyp.tile([P, 512], F32, tag="py", name="py0")
                    py1 = pyp.tile([P, 512], F32, tag="py", name="py1")
                    for j in range(NH):
                        nc.tensor.matmul(py0[:], hid[:, tt, j, :],
                                         wdt[:, 0, j, :],
                                         start=(j == 0), stop=(j == NH - 1))
                        nc.tensor.matmul(py1[:], hid[:, tt, j, :],
                                         wdt[:, 1, j, :],
                                         start=(j == 0), stop=(j == NH - 1))
                    for dh, py in ((0, py0), (1, py1)):
                        ob = op.tile([P, 512], BF16, tag="ob")
                        nc.scalar.mul(ob[:], py[:],
                                      wsc_tile[:, abs_tt:abs_tt + 1])
                        nc.sync.dma_start(
                            y.ap()[abs_tt * P:(abs_tt + 1) * P,
                                   dh * 512:(dh + 1) * 512], ob[:])

            def phase2():
                # overflow tokens (exact counts, no slot padding), all
                # experts, H/8 row-slice per core.  gate/up chains
                # interleave across experts (pairs) so each LDWEIGHTS
                # hides behind the other expert's stream.
                x2c = xp.tile([P, ND, C2], BF16, tag="xc", name="x2c")
                nc.gpsimd.dma_start(x2c[:], x2_t.ap())
                h2 = hp.tile([P, NTL2, NJ2, P], BF16, tag="hid", name="h2")
                # gate/up chunks of <=512 tokens, tile-aligned per expert
                chunks = []
                for gi, (oe, off, n) in enumerate(ovf):
                    t = 0
                    while t < n:
                        cn = min(512, n - t)
                        k0 = next(k for k, (g, o, _n) in enumerate(tiles2)
                                  if g == gi and o == off + t)
                        chunks.append((gi, off + t, cn, k0))
                        t += cn
                for g0 in range(0, len(chunks), 2):
                    grp = chunks[g0:g0 + 2]
                    for j2 in range(NJ2):
                        sil = None
                        for gu, pool in ((0, pgp), (1, pup)):
                            ps = [pool.tile([P, 512], F32, tag=pool.name,
                                            name=f"{pool.name}2_{gi}")
                                  for gi in range(len(grp))]
                            for d in range(ND):
                                for ci, (gi, off, cn, k0) in enumerate(grp):
                                    nc.tensor.matmul(
                                        ps[ci][:, :cn],
                                        w2gut[:, gi, j2, gu, d, :],
                                        x2c[:, d, off:off + cn],
                                        start=(d == 0), stop=(d == ND - 1))
                            if gu == 0:
                                sil = ps
                            else:
                                for ci, (gi, off, cn, k0) in enumerate(grp):
                                    t = 0
                                    k = k0
                                    while t < cn:
                                        tn = min(P, cn - t)
                                        nc.scalar.activation(
                                            h2[:, k, j2, :tn],
                                            sil[ci][:, t:t + tn],
                                            mybir.ActivationFunctionType.Silu)
                                        nc.vector.tensor_mul(
                                            h2[:, k, j2, :tn],
                                            h2[:, k, j2, :tn],
                                            ps[ci][:, t:t + tn])
                                        t += tn
                                        k += 1
                for k, (gi, off, tn) in enumerate(tiles2):
                    py0 = pyp.tile([P, 512], F32, tag="py", name="py0")
                    py1 = pyp.tile([P, 512], F32, tag="py", name="py1")
                    for j2 in range(NJ2):
                        nc.tensor.matmul(py0[:tn, :], h2[:, k, j2, :tn],
                                         w2dt[:, gi, j2, 0, :],
                                         start=(j2 == 0),
                                         stop=(j2 == NJ2 - 1))
                        nc.tensor.matmul(py1[:tn, :], h2[:, k, j2, :tn],
                                         w2dt[:, gi, j2, 1, :],
                                         start=(j2 == 0),
                                         stop=(j2 == NJ2 - 1))
                    for dh, py in ((0, py0), (1, py1)):
                        ob2 = op.tile([P, 512], F32, tag="ob2")
                        nc.scalar.mul(ob2[:tn, :], py[:tn, :],
                                      wsc2_tile[:tn, k:k + 1])
                        nc.sync.dma_start(
                            y2.ap()[off:off + tn,
                                    dh * 512:(dh + 1) * 512], ob2[:tn, :])

            def body():
                # hid[p, tt, j, q] = silu(g)*u for h-row j*128+p, token
                # tt*128+q; down chains read j-ascending contiguous 256B
                # runs within the tt window.  Pair blocks so each gate/up
                # stationary feeds two matmuls (halves LDWEIGHTS traffic).
                i = 0
                while i < len(blocks):
                    grp = blocks[i:i + 2]
                    xcs = [load_x(boff, bs) for (boff, bs) in grp]
                    hds = [hp.tile([P, 4, NH, P], BF16, tag="hid",
                                   name=f"hid{boff}")
                           for (boff, bs) in grp]
                    gateup_grp(grp, xcs, hds)
                    for gi, (boff, bs) in enumerate(grp):
                        down(hds[gi], boff, bs // P)
                    i += 2
                if C2:
                    phase2()

            if repeat == 1:
                body()
            else:
                with tc.For_i(0, repeat, 1):
                    body()
    nc.compile()
    return nc


_NC_CACHE = {}


def get_nc(key, repeat=1):
    ck = (key, repeat)
    if ck not in _NC_CACHE:
        _NC_CACHE[ck] = build_nc(key, repeat)
    return _NC_CACHE[ck]


def route_and_shard(hidden_states, router_w, w_gate, w_up, w_down):
    """Host-side router + per-expert gather. Returns in_maps, combine
    metadata, and the program shape key."""
    import ml_dtypes
    bf16 = ml_dtypes.bfloat16

    x = np.ascontiguousarray(hidden_states.reshape(T, D).astype(np.float32))
    logits = x @ router_w.T.astype(np.float32)              # [T, E]
    # top-2 (no softmax needed: renormalized top-k softmax weights are
    # exp(l_i - m) / sum_topk exp(l - m), selection by logit order)
    order = np.argsort(logits, axis=1)
    top1 = order[:, -1]
    top2 = order[:, -2]
    l1 = logits[np.arange(T), top1]
    l2 = logits[np.arange(T), top2]
    e2 = np.exp(l2 - l1)
    w1 = 1.0 / (1.0 + e2)
    w2 = e2 / (1.0 + e2)

    sel = np.zeros((T, E), dtype=bool)
    sel[np.arange(T), top1] = True
    sel[np.arange(T), top2] = True
    wfull = np.zeros((T, E), dtype=np.float32)
    wfull[np.arange(T), top1] = w1
    wfull[np.arange(T), top2] = w2

    idx_list = [np.nonzero(sel[:, e])[0] for e in range(E)]
    loads = [len(i) for i in idx_list]
    maxload = max(loads)
    maxload_r = ((maxload + P - 1) // P) * P

    # pick phase-1 capacity minimizing the per-core PE column count:
    # C1*384 (phase 1) + ovf_tokens*32 (phase-2 gate/up, exact) +
    # ovf_tiles*2048 (phase-2 down, per-tile quantum) + a small bias for
    # the per-overflow-expert LDWEIGHTS overhead of short chains.
    def cols(c1):
        ot = sum(max(0, L - c1) for L in loads)
        otl = sum((max(0, L - c1) + P - 1) // P for L in loads)
        ne = sum(1 for L in loads if L > c1)
        return c1 * 384 + ot * 32 + otl * 2048 + ne * 1536
    C1 = min(range(P, maxload_r + 1, P), key=cols)
    slots = tuple(max(0, L - C1) for L in loads)
    C2 = sum(slots)
    if C2 > 2048:   # pathological imbalance: fall back to expert-parallel
        C1 = maxload_r
        slots = (0,) * E
        C2 = 0
    NT1 = C1 // P
    tiles2 = []
    ovf_e = [e for e in range(E) if slots[e]]
    off = 0
    for e in ovf_e:
        t = 0
        while t < slots[e]:
            tiles2.append((e, off + t, min(P, slots[e] - t)))
            t += P
        off += slots[e]
    NTL2 = len(tiles2)

    def pm_gateup(wg_e, wu_e, nh):
        # [p, j, gu, d, q] = w{g,u}[j*128+q, d*128+p], j in [0, nh)
        wg_pm = wg_e.T.astype(bf16).reshape(ND, P, nh, P).transpose(1, 2, 0, 3)
        wu_pm = wu_e.T.astype(bf16).reshape(ND, P, nh, P).transpose(1, 2, 0, 3)
        return np.ascontiguousarray(np.stack([wg_pm, wu_pm], axis=2))

    in_maps = []
    for m in range(E):
        idx = idx_list[m][:C1]
        L = len(idx)
        # xt[p, d, c] = x[idx[c], d*128+p]
        xt = np.zeros((P, ND, C1), dtype=bf16)
        xTe = np.ascontiguousarray(x[idx].T).astype(bf16)   # [D, L]
        xt[:, :, :L] = xTe.reshape(ND, P, L).transpose(1, 0, 2)
        ws = np.zeros((P, NT1), dtype=np.float32)
        wflat = np.zeros(C1, dtype=np.float32)
        wflat[:L] = wfull[idx, m]
        ws[:, :] = wflat.reshape(NT1, P).T
        wgu = pm_gateup(w_gate[m], w_up[m], NH)             # [P,NH,2,ND,P]
        # wdp[p, dh, j, c] = w_down[m][dh*512+c, j*128+p]
        wdp = np.ascontiguousarray(
            w_down[m].T.astype(bf16).reshape(NH, P, 2, 512)
            .transpose(1, 2, 0, 3))                         # [P,2,NH,512]
        in_maps.append({
            "x_t": xt,
            "wgu_t": wgu,
            "wd_t": wdp,
            "wsc": ws,
        })

    if C2:
        NO = len(ovf_e)
        x2 = np.zeros((P, ND, C2), dtype=bf16)
        ws2flat = np.zeros(C2, dtype=np.float32)
        off = 0
        for e in ovf_e:
            idx = idx_list[e][C1:]
            L = len(idx)
            xTe = np.ascontiguousarray(x[idx].T).astype(bf16)
            x2[:, :, off:off + L] = xTe.reshape(ND, P, L).transpose(1, 0, 2)
            ws2flat[off:off + L] = wfull[idx, e]
            off += slots[e]
        # per-down-tile per-partition scale column
        ws2 = np.zeros((P, NTL2), dtype=np.float32)
        for k, (e, toff, tn) in enumerate(tiles2):
            ws2[:tn, k] = ws2flat[toff:toff + tn]
        for m in range(N_CORES):
            r0, r1 = m * HS, (m + 1) * HS
            w2gu = np.stack([pm_gateup(w_gate[e][r0:r1], w_up[e][r0:r1], NJ2)
                             for e in ovf_e])               # [NO,P,NJ2,2,ND,P]
            w2gu = np.ascontiguousarray(w2gu.transpose(1, 0, 2, 3, 4, 5))
            # w2d[p, oe, j2, dh, c] = w_down[e][dh*512+c, r0 + j2*128 + p]
            w2d = np.stack([
                np.ascontiguousarray(
                    w_down[e][:, r0:r1].T.astype(bf16)
                    .reshape(NJ2, P, 2, 512))
                for e in ovf_e])                            # [NO,NJ2,P,2,512]
            w2d = np.ascontiguousarray(w2d.transpose(2, 0, 1, 3, 4))
            in_maps[m]["x2_t"] = x2
            in_maps[m]["w2gu"] = w2gu
            in_maps[m]["w2d"] = w2d
            in_maps[m]["wsc2"] = ws2

    meta = (idx_list, C1, slots)
    return in_maps, meta, (C1, slots)


def combine_outputs(results, meta):
    idx_list, C1, slots = meta
    out = np.zeros((T, D), dtype=np.float32)
    for e in range(E):
        idx = idx_list[e][:C1]
        out[idx] += results[e]["y"][:len(idx)].astype(np.float32)
    if sum(slots):
        y2 = np.zeros(results[0]["y2"].shape, dtype=np.float32)
        for m in range(N_CORES):
            y2 += results[m]["y2"]
        off = 0
        for e in range(E):
            if not slots[e]:
                continue
            idx = idx_list[e][C1:]
            out[idx] += y2[off:off + len(idx)]
            off += slots[e]
    return out.reshape(B, S, D)


def kernel(hidden_states, router_w, w_gate, w_up, w_down):
    hidden_states = np.asarray(hidden_states)
    router_w = np.asarray(router_w)
    w_gate = np.asarray(w_gate)
    w_up = np.asarray(w_up)
    w_down = np.asarray(w_down)
    in_maps, meta, key = route_and_shard(
        hidden_states, router_w, w_gate, w_up, w_down)
    nc = get_nc(key)
    last_err = None
    for _attempt in range(3):
        try:
            res = run_bass_kernel_spmd(nc, in_maps, list(range(N_CORES)))
            break
        except Exception as e:  # transient NRT device errors recover on retry
            last_err = e
    else:
        raise last_err
    return combine_outputs(res.results, meta)
